# revision 34
# baseline (speedup 1.0000x reference)
"""Distributed kNN classifier for Trainium2 (8 NeuronCores).

Strategy
--------
reference(...) computes sim = feature @ feature_bank  [B, N], takes top-k
(k=200) per query, exp(sim/0.1) weights, scatter-adds into per-class scores
and returns the descending stable argsort of those scores.

The heavy part is the [1024, 1024] @ [1024, 100000] matmul plus top-k.
feature_bank is sharded along N across the 8 cores (12500 cols each).

Device (default, fp8): each core computes its sim shard with an fp8e4m3
DoubleRow matmul (fp32 PSUM accumulation, 2 MACs/cell/cycle) and writes
uint8 `clamp(round(sim - 64), 0, 255)` — candidate mask and coarse value in
one byte.  Sims are ~N(0, 32^2); every query's true 200th-largest sim is
>= ~84, and the fp8 matmul error is bounded by E_FP8, so the candidates
with stored value >= 10 (sim >~ 74) are a guaranteed superset of the true
top-k.  The host then (a) keeps, per query, only candidates
within 2*E of the device-value 200th-largest (a confidence window that
provably contains the true top-k), (b) recomputes exact fp32 similarities
for those ~0.4% of pairs, (c) selects the exact top-k with jax.lax.top_k
tie semantics and replicates the reference's exp/scatter/argsort in numpy.
If any query yields fewer than k candidates, the host falls back to an
exact full-row recompute for it, so correctness never depends on the
threshold.

Device-kernel layout: inputs are host-packed so d-row r lives on
partition r%128, plane r//128.  featT is two [128, 8, 512] query-halves
(one DMA each on the sync HWDGE ring); the bank shard is loaded in
ramp-up chunks (256/512/1024 then 2048-wide, one DMA per chunk on the
scalar HWDGE ring) so the first matmul only gates on ~0.75 MiB.  The
first two chunks are duplicated in a contiguous-per-partition side
input (bank8head) for wide DMA lines; later chunks read the
plane-strided bank8 so their packets stay small and never starve the
featT transfers.  During the DMA prologue, N_WARM_MM dummy matmuls on
raw uninitialized SBUF keep the PE busy through the HAM activity window
(cold K=4/8 -> warm K=8/8), so the real MM stream runs at the warm
216 ns/MM roofline pace from its first instruction.  The final subtile
uses separate per-query-half PSUM/val tiles and stores on both HWDGE
rings so its qh0 postprocessing drains while the qh1 matmuls still run.
Measured: ~169.5 us of back-to-back matmuls at the fp8 DoubleRow
roofline + ~13 us DMA-gated start + ~12 us fixed framework
preamble/teardown => ~195 us/core (baseline was ~202 us).

A bf16 variant (KNN_IMPL=bf16) with a uint8 `sim > T0` mask output is kept
as a fallback, as is the unpacked fp8 layout (KNN_IMPL=fp8v0).
"""

import os
import sys
import time
import numpy as np
import ml_dtypes


def _tlog(msg, _t=[None]):
    if os.environ.get("KNN_TIMING"):
        now = time.time()
        dt = 0.0 if _t[0] is None else now - _t[0]
        _t[0] = now
        print(f"[knn +{dt:6.2f}s] {msg}", file=sys.stderr, flush=True)


import concourse.bass as bass
import concourse.bacc as bacc
import concourse.mybir as mybir
from concourse import tile
from concourse.bass_utils import run_bass_kernel_spmd

# Problem geometry (hardcoded per spec).
B = 1024          # queries
D = 1024          # feature dim
N_TOTAL = 100000  # bank size
N_CORES = 8
N_SHARD = N_TOTAL // N_CORES  # 12500

P = 128           # partitions
KCH = D // P      # 8 contraction planes (one per 128 d-rows)
KK = D // (2 * P)  # 4 DoubleRow contraction chunks (fp8)
QW = 512          # rhs free width per matmul (one PSUM bank of fp32)
CH = 512          # bank columns loaded per DMA chunk (bf16 path)

T0 = 80.0         # bf16 mask threshold (true 200th-largest sim is >= ~84.2)
VAL_OFF = 64.0    # u8 value-output offset: stored = clamp(sim - 64, 0, 255)
T0_FP8_U8 = 10    # u8 threshold (sim >~ 74; fp8 |err| <= ~6.6, margin ~10)
E_FP8 = 8.5       # fp8 matmul + u8 quantization error bound for the window

# fp8 kernel chunking: ramp-up chunks so matmuls start as soon as ~0.75 MiB
# has landed and each later chunk arrives before compute reaches it
# (sum == N_SHARD).
CHUNKS_V2 = (256, 512, 1024, 2048, 2048, 2048, 2048, 2048, 468)
CH_ALLOC_V2 = 2048  # SBUF tile width for big chunks (plane stride, 16-aligned)
N_WARM_MM = 12      # dummy MMs to warm the PE HAM gate during the DMA prologue

KNN_T = 0.1

LAST_EXEC_TIME_NS = None
LAST_DEV_VALS = None  # [N, B] device sims (fp8 path), for diagnostics


def _build_program_bf16(n_shard: int = N_SHARD):
    """bf16 matmul; uint8 mask output."""
    nc = bacc.Bacc("TRN2", target_bir_lowering=False, debug=False)

    featT = nc.dram_tensor("featT", [D, B], mybir.dt.bfloat16, kind="ExternalInput")
    bank = nc.dram_tensor("bank", [D, n_shard], mybir.dt.bfloat16, kind="ExternalInput")
    mask = nc.dram_tensor("mask", [n_shard, B], mybir.dt.uint8, kind="ExternalOutput")

    with tile.TileContext(nc) as tc:
        with (
            tc.tile_pool(name="feat", bufs=1) as feat_pool,
            tc.tile_pool(name="bankp", bufs=4) as bank_pool,
            tc.tile_pool(name="maskp", bufs=6) as mask_pool,
            tc.tile_pool(name="psum", bufs=6, space=bass.MemorySpace.PSUM) as psum_pool,
        ):
            # All of feature^T stays resident: [128, 8, 1024] bf16 (16 KiB/part)
            featT_sb = feat_pool.tile([P, KCH, B], mybir.dt.bfloat16)
            for kc in range(KCH):
                nc.sync.dma_start(featT_sb[:, kc, :], featT[kc * P:(kc + 1) * P, :])

            nch = (n_shard + CH - 1) // CH
            for ci in range(nch):
                c0 = ci * CH
                cw = min(CH, n_shard - c0)
                bank_sb = bank_pool.tile([P, KCH, CH], mybir.dt.bfloat16)
                for kc in range(KCH):
                    nc.sync.dma_start(
                        bank_sb[:, kc, :cw], bank[kc * P:(kc + 1) * P, c0:c0 + cw]
                    )
                for si in range(0, cw, P):
                    sw = min(P, cw - si)
                    mask_t = mask_pool.tile([P, B], mybir.dt.uint8)
                    for qh in range(B // QW):
                        ps = psum_pool.tile([P, QW], mybir.dt.float32)
                        for kc in range(KCH):
                            nc.tensor.matmul(
                                ps[:sw, :],
                                bank_sb[:, kc, si:si + sw],
                                featT_sb[:, kc, qh * QW:(qh + 1) * QW],
                                start=(kc == 0),
                                stop=(kc == KCH - 1),
                            )
                        nc.vector.tensor_scalar(
                            out=mask_t[:sw, qh * QW:(qh + 1) * QW],
                            in0=ps[:sw, :],
                            scalar1=T0,
                            scalar2=None,
                            op0=mybir.AluOpType.is_gt,
                        )
                    nc.sync.dma_start(mask[c0 + si:c0 + si + sw, :], mask_t[:sw, :])

    nc.compile()
    return nc


def _build_program_fp8(n_shard: int = N_SHARD):
    """fp8e4m3 DoubleRow matmul, host-packed layout (v2).

    Inputs (packed on host; d-row r -> partition r % 128, plane r // 128):
      featT8 [128, 2, 8, 512]: featT8[p, qh, j, c] = feature[512*qh + c, 128*j + p]
      bank8  [128, 8 * n_shard]: chunk-major — for chunk (c0, cw), partition p
        holds the chunk's 8 planes contiguously ([8, cw] row-major), so each
        chunk is one contiguous-per-partition DMA with >= 2 KiB lines.
    Output: vals [n_shard, B] u8 = clamp(round(sim - VAL_OFF), 0, 255).
    """
    assert n_shard == sum(CHUNKS_V2)
    nc = bacc.Bacc("TRN2", target_bir_lowering=False, debug=False)

    featT8 = nc.dram_tensor(
        "featT8", [P, 2, KCH, QW], mybir.dt.float8e4, kind="ExternalInput"
    )
    bank8 = nc.dram_tensor(
        "bank8", [P, KCH, n_shard], mybir.dt.float8e4, kind="ExternalInput"
    )
    head_cols = CHUNKS_V2[0] + CHUNKS_V2[1]
    bank8head = nc.dram_tensor(
        "bank8head", [P, KCH * head_cols], mybir.dt.float8e4,
        kind="ExternalInput",
    )
    vals = nc.dram_tensor("vals", [n_shard, B], mybir.dt.uint8, kind="ExternalOutput")

    DR = mybir.MatmulPerfMode.DoubleRow

    # PE warmup operands outside the Tile world: raw (uninitialized) SBUF as
    # garbage source, and a PSUM bank that is freed back to the stack before
    # the tile pools claim all 8 banks.  Safe because the in-order PE queue
    # runs every warmup MM before the first real MM, whose start=True clears
    # the bank's has_written state.
    wsrc = nc.alloc_sbuf_tensor("warm_src", [P, 2, QW], mybir.dt.float8e4)
    wsrc_ap = wsrc[:, :, :]

    # Gate-critical loads issued as raw bass BEFORE the TileContext entry
    # barrier, so their ring transfers start ~1.3 us earlier than any
    # tile-managed DMA could: featT halves on the sync ring, chunk0 on the
    # scalar ring.  Completion is signalled on manual semaphores; the waits
    # are attached to the first consuming matmuls AFTER the Tile scheduler
    # runs (it cannot model producers outside its block).
    cw0, cw1 = CHUNKS_V2[0], CHUNKS_V2[1]
    fq0_r = nc.alloc_sbuf_tensor("fq0_r", [P, KCH, QW], mybir.dt.float8e4)
    fq1_r = nc.alloc_sbuf_tensor("fq1_r", [P, KCH, QW], mybir.dt.float8e4)
    c0_r = nc.alloc_sbuf_tensor("c0_r", [P, KCH, cw0], mybir.dt.float8e4)
    s_fq0 = nc.alloc_semaphore("s_fq0")
    s_fq1 = nc.alloc_semaphore("s_fq1")
    s_c0 = nc.alloc_semaphore("s_c0")
    nc.sync.dma_start(fq0_r[:, :, :], featT8[:, 0, :, :]).then_inc(s_fq0, 16)
    nc.sync.dma_start(fq1_r[:, :, :], featT8[:, 1, :, :]).then_inc(s_fq1, 16)
    nc.scalar.dma_start(
        c0_r[:, :, :], bank8head[:, 0:KCH * cw0]
    ).then_inc(s_c0, 16)
    # Placeholder waits (sem >= 0, trivially satisfied so the Tile
    # scheduler's simulator never blocks on them); upgraded to >= 16 after
    # scheduling.  In-order engine queues make one wait per sem sufficient.
    gate_waits = []  # (BassInstruction placeholder, sem)
    defer_dmas = []  # chunk-prefetch DMAs held until the gate completes

    with tile.TileContext(nc) as tc:
        with nc.psum_tensor([P, QW], mybir.dt.float32) as wps:
            wps_ap = wps[:, :]
            for _ in range(N_WARM_MM):
                nc.tensor.matmul(
                    wps_ap, wsrc_ap[:, :, 0:P], wsrc_ap,
                    start=True, stop=True, perf_mode=DR,
                )
            gate_waits.append((nc.tensor.wait_ge(s_c0, 0), s_c0))
            gate_waits.append((nc.tensor.wait_ge(s_fq0, 0), s_fq0))
        with (
            tc.tile_pool(name="feat", bufs=1) as feat_pool,
            tc.tile_pool(name="b256", bufs=1) as b256_pool,
            tc.tile_pool(name="b512", bufs=1) as b512_pool,
            tc.tile_pool(name="b1024", bufs=1) as b1024_pool,
            tc.tile_pool(name="bankp", bufs=3) as bank_pool,
            tc.tile_pool(name="valp", bufs=6) as val_pool,
            tc.tile_pool(name="psum", bufs=4, space=bass.MemorySpace.PSUM) as psum_pool,
        ):
            # All loads go on the sync HWDGE ring in gate-priority order
            # (FIFO per ring): featT qh0, bank chunk0, featT qh1, chunks 1+.
            # Stores use the scalar ring so they never queue ahead of loads.
            # Distinct callsites: same-line pool.tile() calls share a tag and
            # would alias the same buffer (deadlock).
            small_pools = {256: b256_pool, 512: b512_pool, 1024: b1024_pool}
            bank_tiles = {}

            def new_ps():
                # Shared callsite -> one pool tag -> one rotating buffer set.
                ps = psum_pool.tile([P, B], mybir.dt.float32)
                return ps

            def new_val():
                val_t = val_pool.tile([P, B], mybir.dt.uint8)
                return val_t

            bank_tiles[0] = c0_r
            featq = [fq0_r, fq1_r]

            bank_c1 = small_pools[cw1].tile([P, KCH, cw1], mybir.dt.float8e4)
            bank_tiles[1] = bank_c1
            defer_dmas.append(nc.scalar.dma_start(
                bank_c1[:, :, :], bank8head[:, KCH * cw0:KCH * (cw0 + cw1)]
            ))

            n_subtiles = sum((cw + P - 1) // P for cw in CHUNKS_V2)
            sub_i = 0
            c0 = 0
            for ci, cw in enumerate(CHUNKS_V2):
                if ci <= 1:
                    bank_sb = bank_tiles[ci]
                else:
                    if ci < 3:
                        ca = cw
                        pool = small_pools[cw]
                    else:
                        ca = CH_ALLOC_V2
                        pool = bank_pool
                    bank_sb = pool.tile([P, KCH, ca], mybir.dt.float8e4)
                    # One DMA per chunk (plane-strided source); the HW
                    # splits it across all 16 SDMA engines.
                    dma = nc.scalar.dma_start(
                        bank_sb[:, :, :cw], bank8[:, :, c0:c0 + cw]
                    )
                    if ci == 2:
                        defer_dmas.append(dma)
                for si in range(0, cw, P):
                    sw = min(P, cw - si)
                    sub_i += 1
                    if sub_i < n_subtiles:
                        ps = new_ps()
                        val_t = new_val()
                        for qh in range(2):
                            if ci == 0 and si == 0 and qh == 1:
                                gate_waits.append(
                                    (nc.tensor.wait_ge(s_fq1, 0), s_fq1)
                                )
                            for kk in range(KK):
                                nc.tensor.matmul(
                                    ps[:sw, qh * QW:(qh + 1) * QW],
                                    bank_sb[:, 2 * kk:2 * kk + 2, si:si + sw],
                                    featq[qh][:, 2 * kk:2 * kk + 2, :],
                                    start=(kk == 0),
                                    stop=(kk == KK - 1),
                                    perf_mode=DR,
                                )
                        # (sim - 64) clamped at 0, cast to u8 — single DVE op.
                        nc.vector.tensor_scalar(
                            out=val_t[:sw, :],
                            in0=ps[:sw, :],
                            scalar1=-VAL_OFF,
                            scalar2=0.0,
                            op0=mybir.AluOpType.add,
                            op1=mybir.AluOpType.max,
                        )
                        nc.sync.dma_start(
                            vals[c0 + si:c0 + si + sw, :], val_t[:sw, :]
                        )
                    else:
                        # Final subtile: separate psum tiles per query half so
                        # the qh0 DVE+store drain while the qh1 MMs still run
                        # (a shared tile would add a whole-tile WAR edge).
                        ps_a = new_ps()
                        ps_b = new_ps()
                        val_a = new_val()
                        val_b = new_val()
                        for qh, ps_h, val_h in ((0, ps_a, val_a), (1, ps_b, val_b)):
                            for kk in range(KK):
                                nc.tensor.matmul(
                                    ps_h[:sw, 0:QW],
                                    bank_sb[:, 2 * kk:2 * kk + 2, si:si + sw],
                                    featq[qh][:, 2 * kk:2 * kk + 2, :],
                                    start=(kk == 0),
                                    stop=(kk == KK - 1),
                                    perf_mode=DR,
                                )
                            qs = slice(qh * QW, (qh + 1) * QW)
                            nc.vector.tensor_scalar(
                                out=val_h[:sw, 0:QW], in0=ps_h[:sw, 0:QW],
                                scalar1=-VAL_OFF, scalar2=0.0,
                                op0=mybir.AluOpType.add, op1=mybir.AluOpType.max,
                            )
                            eng = nc.scalar if qh == 0 else nc.sync
                            eng.dma_start(
                                vals[c0 + si:c0 + si + sw, qs], val_h[:sw, 0:QW]
                            )
                c0 += cw

    # Upgrade the placeholder waits now that the Tile scheduler has run (it
    # cannot model producers outside its block and would report a deadlock).
    for w, sem in gate_waits:
        w.wait_op(sem, 16, "sem-ge")
    # Hold the chunk1/chunk2 prefetches until the gate set has landed: the
    # two NeuronCores of an HBM stack share ~716 GB/s, so an eager prefetch
    # on one core starves its neighbour's gate transfers (seen as a +3 us
    # late start).  Later chunks queue behind chunk2 on the scalar ring.
    for dma in defer_dmas:
        dma.wait_op(s_c0, 16, "sem-ge")

    nc.compile()
    return nc


def _build_program_fp8_v0(n_shard: int = N_SHARD):
    """Previous fp8 layout (unpacked inputs, 8 DMAs per chunk). Fallback."""
    nc = bacc.Bacc("TRN2", target_bir_lowering=False, debug=False)

    featT8 = nc.dram_tensor("featT8", [D, B], mybir.dt.float8e4, kind="ExternalInput")
    bank8 = nc.dram_tensor("bank8", [D, n_shard], mybir.dt.float8e4, kind="ExternalInput")
    vals = nc.dram_tensor("vals", [n_shard, B], mybir.dt.uint8, kind="ExternalOutput")

    CH8 = 1024  # bank cols per DMA chunk (1 KiB fp8 rows)
    with tile.TileContext(nc) as tc:
        with (
            tc.tile_pool(name="feat", bufs=1) as feat_pool,
            tc.tile_pool(name="bankp", bufs=4) as bank_pool,
            tc.tile_pool(name="valp", bufs=8) as val_pool,
            tc.tile_pool(name="psum", bufs=4, space=bass.MemorySpace.PSUM) as psum_pool,
        ):
            featT_sb = feat_pool.tile([P, KK, 2, B], mybir.dt.float8e4)
            for kk in range(KK):
                for i in range(2):
                    r0 = (2 * kk + i) * P
                    nc.sync.dma_start(featT_sb[:, kk, i, :], featT8[r0:r0 + P, :])

            nch = (n_shard + CH8 - 1) // CH8
            for ci in range(nch):
                c0 = ci * CH8
                cw = min(CH8, n_shard - c0)
                bank_sb = bank_pool.tile([P, KK, 2, CH8], mybir.dt.float8e4)
                for kk in range(KK):
                    for i in range(2):
                        r0 = (2 * kk + i) * P
                        nc.sync.dma_start(
                            bank_sb[:, kk, i, :cw], bank8[r0:r0 + P, c0:c0 + cw]
                        )
                for si in range(0, cw, P):
                    sw = min(P, cw - si)
                    val_t = val_pool.tile([P, B], mybir.dt.uint8)
                    ps = psum_pool.tile([P, B], mybir.dt.float32)  # 2 PSUM banks
                    for qh in range(B // QW):
                        for kk in range(KK):
                            nc.tensor.matmul(
                                ps[:sw, qh * QW:(qh + 1) * QW],
                                bank_sb[:, kk, :, si:si + sw],
                                featT_sb[:, kk, :, qh * QW:(qh + 1) * QW],
                                start=(kk == 0),
                                stop=(kk == KK - 1),
                                perf_mode=mybir.MatmulPerfMode.DoubleRow,
                            )
                    nc.vector.tensor_scalar(
                        out=val_t[:sw, :],
                        in0=ps[:sw, :],
                        scalar1=-VAL_OFF,
                        scalar2=0.0,
                        op0=mybir.AluOpType.add,
                        op1=mybir.AluOpType.max,
                    )
                    nc.sync.dma_start(vals[c0 + si:c0 + si + sw, :], val_t[:sw, :])

    nc.compile()
    return nc


_PROGRAM_CACHE = {}


def _get_program(impl, n_shard):
    key = (impl, n_shard)
    if key not in _PROGRAM_CACHE:
        build = {
            "fp8": _build_program_fp8,
            "fp8v0": _build_program_fp8_v0,
            "bf16": _build_program_bf16,
        }[impl]
        _PROGRAM_CACHE[key] = build(n_shard)
    return _PROGRAM_CACHE[key]


def _profile_ctx():
    import contextlib

    @contextlib.contextmanager
    def _maybe_profile():
        """Optional NTFF capture via the axon NRT-profile C ABI."""
        prof_dir = os.environ.get("KNN_PROFILE_DIR")
        if not prof_dir:
            yield
            return
        import ctypes
        lib = ctypes.CDLL("/opt/axon/libaxon_pjrt.so")
        lib.axon_start_nrt_profile.argtypes = [
            ctypes.POINTER(ctypes.c_int64), ctypes.c_size_t]
        lib.axon_start_nrt_profile.restype = ctypes.c_int64
        lib.axon_stop_nrt_profile.argtypes = [ctypes.c_char_p]
        lib.axon_stop_nrt_profile.restype = ctypes.c_int64
        import jax
        jax.devices()
        rc = lib.axon_start_nrt_profile(None, 0)
        if rc != 0:
            raise RuntimeError(f"axon_start_nrt_profile rc={rc}")
        try:
            yield
        finally:
            n = lib.axon_stop_nrt_profile(str(prof_dir).encode())
            print(f"ntff profile: {n} file(s) -> {prof_dir}", flush=True)

    return _maybe_profile()


def _run_spmd(nc, in_maps):
    global LAST_EXEC_TIME_NS
    with _profile_ctx():
        res = run_bass_kernel_spmd(
            nc, in_maps, core_ids=list(range(N_CORES)), trace=False
        )
    LAST_EXEC_TIME_NS = res.exec_time_ns
    _tlog("device run done")
    return res


def _candidate_pairs_bf16(feature, bank_f32):
    """bf16+mask path: device mask -> all candidate pairs."""
    n = bank_f32.shape[1]
    n_shard = n // N_CORES
    nc = _get_program("bf16", n_shard)
    _tlog("program built")

    featT_bf = np.ascontiguousarray(feature.T).astype(ml_dtypes.bfloat16)
    bank_bf = bank_f32.astype(ml_dtypes.bfloat16)
    in_maps = [
        {
            "featT": featT_bf,
            "bank": np.ascontiguousarray(bank_bf[:, i * n_shard:(i + 1) * n_shard]),
        }
        for i in range(N_CORES)
    ]
    res = _run_spmd(nc, in_maps)
    mask = np.concatenate([res.results[i]["mask"] for i in range(N_CORES)], axis=0)

    nidx, qidx = np.nonzero(mask)  # [N, B]: sorted by bank idx
    order = np.argsort(qidx, kind="stable")  # per-query segments, nidx ascending
    qidx = qidx[order]
    nidx = nidx[order]
    counts = np.bincount(qidx, minlength=feature.shape[0])
    starts = np.zeros(feature.shape[0] + 1, dtype=np.int64)
    np.cumsum(counts, out=starts[1:])
    _tlog(f"candidates built ({len(nidx)} pairs)")
    return qidx, nidx, starts


def _candidate_pairs_fp8(feature, bank_f32, k, impl="fp8"):
    """fp8+values path: threshold, then keep only the top-k confidence window."""
    global LAST_DEV_VALS
    n = bank_f32.shape[1]
    n_shard = n // N_CORES
    nc = _get_program(impl, n_shard)
    _tlog("program built")

    f8 = feature.astype(ml_dtypes.float8_e4m3)
    bank_8 = bank_f32.astype(ml_dtypes.float8_e4m3)
    if impl == "fp8":
        # Packed layouts (see _build_program_fp8 docstring).
        featT_pack = np.ascontiguousarray(
            f8.T.reshape(KCH, P, 2, QW).transpose(1, 2, 0, 3)
        )

        def _pack_bank(shard):
            # d-row r -> partition r % 128, plane r // 128.
            return np.ascontiguousarray(
                shard.reshape(KCH, P, n_shard).transpose(1, 0, 2)
            )

        def _pack_bank_head(shard):
            # chunks 0+1 duplicated contiguous-per-partition (big DMA lines).
            cw0, cw1 = CHUNKS_V2[0], CHUNKS_V2[1]
            out = np.empty((P, KCH * (cw0 + cw1)), dtype=shard.dtype)
            out[:, :KCH * cw0] = (
                shard[:, :cw0].reshape(KCH, P, cw0)
                .transpose(1, 0, 2).reshape(P, KCH * cw0)
            )
            out[:, KCH * cw0:] = (
                shard[:, cw0:cw0 + cw1].reshape(KCH, P, cw1)
                .transpose(1, 0, 2).reshape(P, KCH * cw1)
            )
            return out

        in_maps = [
            {
                "featT8": featT_pack,
                "bank8": _pack_bank(bank_8[:, i * n_shard:(i + 1) * n_shard]),
                "bank8head": _pack_bank_head(
                    bank_8[:, i * n_shard:(i + 1) * n_shard]
                ),
            }
            for i in range(N_CORES)
        ]
    else:
        featT_8 = np.ascontiguousarray(f8.T)
        in_maps = [
            {
                "featT8": featT_8,
                "bank8": np.ascontiguousarray(bank_8[:, i * n_shard:(i + 1) * n_shard]),
            }
            for i in range(N_CORES)
        ]
    _tlog("inputs packed")
    res = _run_spmd(nc, in_maps)
    vals = np.concatenate([res.results[i]["vals"] for i in range(N_CORES)], axis=0)
    LAST_DEV_VALS = vals  # [N, B] u8: clamp(sim - VAL_OFF, 0, 255)

    m = vals >= np.uint8(T0_FP8_U8)
    nidx, qidx = np.nonzero(m)
    # The DVE f32->u8 cast rounds-to-nearest, so stored+VAL_OFF is already the
    # quantization-interval midpoint (E_FP8 covers the +-0.5 either way).
    dv = vals[nidx, qidx].astype(np.float32) + np.float32(VAL_OFF)
    order = np.argsort(qidx, kind="stable")  # per-query segments, nidx ascending
    qidx = qidx[order]
    nidx = nidx[order]
    dv = dv[order]
    b = feature.shape[0]
    counts = np.bincount(qidx, minlength=b)
    starts_all = np.zeros(b + 1, dtype=np.int64)
    np.cumsum(counts, out=starts_all[1:])
    _tlog(f"thresholded ({len(nidx)} pairs)")

    # Per query, keep only candidates that can possibly be in the true top-k:
    # dev >= dev_rank_k - 2E (see module docstring for the bound).
    keep = np.zeros(len(nidx), dtype=bool)
    for q in range(b):
        s, e = starts_all[q], starts_all[q + 1]
        c = e - s
        if c < k:
            keep[s:e] = True  # top-k loop will take the full-row fallback
            continue
        seg = dv[s:e]
        rk = np.partition(seg, c - k)[c - k]
        keep[s:e] = seg >= rk - 2.0 * E_FP8
    qidx = qidx[keep]
    nidx = nidx[keep]
    counts = np.bincount(qidx, minlength=b)
    starts = np.zeros(b + 1, dtype=np.int64)
    np.cumsum(counts, out=starts[1:])
    _tlog(f"windowed ({len(nidx)} pairs)")
    return qidx, nidx, starts


def _finish(feature, bank_f32, labels, num_classes, k, cand):
    """Exact fp32 re-rank of candidate pairs + reference post-processing.

    cand is (qidx, nidx, starts) or None (full host fallback).
    """
    b, d = feature.shape
    n = bank_f32.shape[1]

    if cand is not None:
        qidx, nidx, starts = cand
        bankT = np.ascontiguousarray(bank_f32.T)  # contiguous row gathers
        _tlog("bankT transpose done")
        vals = np.empty(len(nidx), dtype=np.float32)
        CHP = 1 << 16
        for s in range(0, len(nidx), CHP):
            e = min(s + CHP, len(nidx))
            vals[s:e] = np.einsum(
                "ij,ij->i", feature[qidx[s:e]], bankT[nidx[s:e]]
            )
        _tlog(f"exact vals done ({len(nidx)} pairs)")

    full_rows = None
    full_q0 = 0
    all_idx = np.arange(n)

    sel_q = np.empty(b * k, dtype=np.int64)
    sel_lab = np.empty(b * k, dtype=np.int64)
    sel_val = np.empty(b * k, dtype=np.float32)
    pos = 0
    ROWBLK = 64
    for q in range(b):
        if cand is not None and starts[q + 1] - starts[q] >= k:
            s, e = starts[q], starts[q + 1]
            v = vals[s:e]
            idx = nidx[s:e]
        else:
            # Exact full row (no device pre-filter, or threshold miss).
            if full_rows is None or not (full_q0 <= q < full_q0 + ROWBLK):
                full_q0 = q
                hi = min(q + ROWBLK, b)
                full_rows = feature[q:hi] @ bank_f32
            v = full_rows[q - full_q0]
            idx = all_idx
        # jax.lax.top_k semantics: descending, ties -> lower index first.
        order = np.argsort(-v, kind="stable")[:k]
        sel_q[pos:pos + k] = q
        sel_lab[pos:pos + k] = labels[idx[order]]
        sel_val[pos:pos + k] = v[order]
        pos += k
    _tlog("per-query topk done")

    with np.errstate(over="ignore"):
        w = np.exp(sel_val / np.float32(KNN_T)).astype(np.float32)
    scores = np.zeros((b, num_classes), dtype=np.float32)
    np.add.at(scores, (sel_q, sel_lab), w)
    _tlog("scatter done")
    return scores


def kernel(feature, feature_bank, feature_labels, num_classes, knn_k):
    _tlog("kernel() start")
    feature = np.asarray(feature, dtype=np.float32)
    bank_f32 = np.asarray(feature_bank, dtype=np.float32)
    labels = np.asarray(feature_labels)
    c = int(np.asarray(num_classes))
    k = int(np.asarray(knn_k))

    b, d = feature.shape
    n = bank_f32.shape[1]

    impl = os.environ.get("KNN_IMPL", "fp8")
    use_device = d == D and b == B and n % N_CORES == 0 and n // N_CORES > 0
    if use_device and impl == "fp8" and n // N_CORES != sum(CHUNKS_V2):
        impl = "fp8v0"  # packed layout is hardcoded for the spec shard size
    if use_device:
        if impl in ("fp8", "fp8v0"):
            cand = _candidate_pairs_fp8(feature, bank_f32, k, impl)
        else:
            cand = _candidate_pairs_bf16(feature, bank_f32)
    else:
        cand = None  # degenerate fallback: host does it all

    scores = _finish(feature, bank_f32, labels, c, k, cand)
    pred = np.argsort(-scores, axis=1, kind="stable").astype(np.int32)
    _tlog("final argsort done")
    return pred


# revision 35
# speedup vs baseline: 1.0199x; 1.0199x over previous
"""Distributed kNN classifier for Trainium2 (8 NeuronCores).

Strategy
--------
reference(...) computes sim = feature @ feature_bank  [B, N], takes top-k
(k=200) per query, exp(sim/0.1) weights, scatter-adds into per-class scores
and returns the descending stable argsort of those scores.

The heavy part is the [1024, 1024] @ [1024, 100000] matmul plus top-k.
feature_bank is sharded along N across the 8 cores (12500 cols each).

Device (default, fp8): each core computes its sim shard with an fp8e4m3
DoubleRow matmul (fp32 PSUM accumulation, 2 MACs/cell/cycle) and writes
uint8 `clamp(round(sim - 64), 0, 255)` — candidate mask and coarse value in
one byte.  Sims are ~N(0, 32^2); every query's true 200th-largest sim is
>= ~84, and the fp8 matmul error is bounded by E_FP8, so the candidates
with stored value >= 10 (sim >~ 74) are a guaranteed superset of the true
top-k.  The host then (a) keeps, per query, only candidates
within 2*E of the device-value 200th-largest (a confidence window that
provably contains the true top-k), (b) recomputes exact fp32 similarities
for those ~0.4% of pairs, (c) selects the exact top-k with jax.lax.top_k
tie semantics and replicates the reference's exp/scatter/argsort in numpy.
If any query yields fewer than k candidates, the host falls back to an
exact full-row recompute for it, so correctness never depends on the
threshold.

Device-kernel layout: inputs are host-packed so d-row r lives on
partition r%128, plane r//128.  featT is two [128, 8, 512] query-halves
(one DMA each on the sync HWDGE ring); the bank shard is loaded in
ramp-up chunks (256/512/1024 then 2048-wide, one DMA per chunk on the
scalar HWDGE ring) so the first matmul only gates on ~0.75 MiB.  The
first two chunks are duplicated in a contiguous-per-partition side
input (bank8head) for wide DMA lines; later chunks read the
plane-strided bank8 so their packets stay small and never starve the
featT transfers.  During the DMA prologue, N_WARM_MM dummy matmuls on
raw uninitialized SBUF keep the PE busy through the HAM activity window
(cold K=4/8 -> warm K=8/8), so the real MM stream runs at the warm
216 ns/MM roofline pace from its first instruction.  The final subtile
uses separate per-query-half PSUM/val tiles and stores on both HWDGE
rings so its qh0 postprocessing drains while the qh1 matmuls still run.
Measured: ~169.5 us of back-to-back matmuls at the fp8 DoubleRow
roofline + ~13 us DMA-gated start + ~12 us fixed framework
preamble/teardown => ~195 us/core (baseline was ~202 us).

A bf16 variant (KNN_IMPL=bf16) with a uint8 `sim > T0` mask output is kept
as a fallback, as is the unpacked fp8 layout (KNN_IMPL=fp8v0).
"""

import os
import sys
import time
import numpy as np
import ml_dtypes


def _tlog(msg, _t=[None]):
    if os.environ.get("KNN_TIMING"):
        now = time.time()
        dt = 0.0 if _t[0] is None else now - _t[0]
        _t[0] = now
        print(f"[knn +{dt:6.2f}s] {msg}", file=sys.stderr, flush=True)


import concourse.bass as bass
import concourse.bacc as bacc
import concourse.mybir as mybir
from concourse import tile
from concourse.bass_utils import run_bass_kernel_spmd

# Problem geometry (hardcoded per spec).
B = 1024          # queries
D = 1024          # feature dim
N_TOTAL = 100000  # bank size
N_CORES = 8
N_SHARD = N_TOTAL // N_CORES  # 12500

P = 128           # partitions
KCH = D // P      # 8 contraction planes (one per 128 d-rows)
KK = D // (2 * P)  # 4 DoubleRow contraction chunks (fp8)
QW = 512          # rhs free width per matmul (one PSUM bank of fp32)
CH = 512          # bank columns loaded per DMA chunk (bf16 path)

T0 = 80.0         # bf16 mask threshold (true 200th-largest sim is >= ~84.2)
VAL_OFF = 64.0    # u8 value-output offset: stored = clamp(sim - 64, 0, 255)
T0_FP8_U8 = 10    # u8 threshold (sim >~ 74; fp8 |err| <= ~6.6, margin ~10)
E_FP8 = 8.5       # fp8 matmul + u8 quantization error bound for the window

# fp8 kernel chunking: ramp-up chunks so matmuls start as soon as ~0.75 MiB
# has landed and each later chunk arrives before compute reaches it
# (sum == N_SHARD).
CHUNKS_V2 = (256, 512, 1024, 2048, 2048, 2048, 2048, 2048, 468)
CH_ALLOC_V2 = 2048  # SBUF tile width for big chunks (plane stride, 16-aligned)
N_WARM_MM = 12      # dummy MMs to warm the PE HAM gate during the DMA prologue

KNN_T = 0.1

LAST_EXEC_TIME_NS = None
LAST_DEV_VALS = None  # [N, B] device sims (fp8 path), for diagnostics


def _build_program_bf16(n_shard: int = N_SHARD):
    """bf16 matmul; uint8 mask output."""
    nc = bacc.Bacc("TRN2", target_bir_lowering=False, debug=False)

    featT = nc.dram_tensor("featT", [D, B], mybir.dt.bfloat16, kind="ExternalInput")
    bank = nc.dram_tensor("bank", [D, n_shard], mybir.dt.bfloat16, kind="ExternalInput")
    mask = nc.dram_tensor("mask", [n_shard, B], mybir.dt.uint8, kind="ExternalOutput")

    with tile.TileContext(nc) as tc:
        with (
            tc.tile_pool(name="feat", bufs=1) as feat_pool,
            tc.tile_pool(name="bankp", bufs=4) as bank_pool,
            tc.tile_pool(name="maskp", bufs=6) as mask_pool,
            tc.tile_pool(name="psum", bufs=6, space=bass.MemorySpace.PSUM) as psum_pool,
        ):
            # All of feature^T stays resident: [128, 8, 1024] bf16 (16 KiB/part)
            featT_sb = feat_pool.tile([P, KCH, B], mybir.dt.bfloat16)
            for kc in range(KCH):
                nc.sync.dma_start(featT_sb[:, kc, :], featT[kc * P:(kc + 1) * P, :])

            nch = (n_shard + CH - 1) // CH
            for ci in range(nch):
                c0 = ci * CH
                cw = min(CH, n_shard - c0)
                bank_sb = bank_pool.tile([P, KCH, CH], mybir.dt.bfloat16)
                for kc in range(KCH):
                    nc.sync.dma_start(
                        bank_sb[:, kc, :cw], bank[kc * P:(kc + 1) * P, c0:c0 + cw]
                    )
                for si in range(0, cw, P):
                    sw = min(P, cw - si)
                    mask_t = mask_pool.tile([P, B], mybir.dt.uint8)
                    for qh in range(B // QW):
                        ps = psum_pool.tile([P, QW], mybir.dt.float32)
                        for kc in range(KCH):
                            nc.tensor.matmul(
                                ps[:sw, :],
                                bank_sb[:, kc, si:si + sw],
                                featT_sb[:, kc, qh * QW:(qh + 1) * QW],
                                start=(kc == 0),
                                stop=(kc == KCH - 1),
                            )
                        nc.vector.tensor_scalar(
                            out=mask_t[:sw, qh * QW:(qh + 1) * QW],
                            in0=ps[:sw, :],
                            scalar1=T0,
                            scalar2=None,
                            op0=mybir.AluOpType.is_gt,
                        )
                    nc.sync.dma_start(mask[c0 + si:c0 + si + sw, :], mask_t[:sw, :])

    nc.compile()
    return nc


def _build_program_fp8(n_shard: int = N_SHARD):
    """fp8e4m3 DoubleRow matmul, host-packed layout (v2).

    Inputs (packed on host; d-row r -> partition r % 128, plane r // 128):
      featT8 [128, 2, 8, 512]: featT8[p, qh, j, c] = feature[512*qh + c, 128*j + p]
      bank8  [128, 8 * n_shard]: chunk-major — for chunk (c0, cw), partition p
        holds the chunk's 8 planes contiguously ([8, cw] row-major), so each
        chunk is one contiguous-per-partition DMA with >= 2 KiB lines.
    Output: vals [n_shard, B] u8 = clamp(round(sim - VAL_OFF), 0, 255).
    """
    assert n_shard == sum(CHUNKS_V2)
    nc = bacc.Bacc("TRN2", target_bir_lowering=False, debug=False)

    featT8 = nc.dram_tensor(
        "featT8", [P, 2, KCH, QW], mybir.dt.float8e4, kind="ExternalInput"
    )
    bank8 = nc.dram_tensor(
        "bank8", [P, KCH, n_shard], mybir.dt.float8e4, kind="ExternalInput"
    )
    head_cols = CHUNKS_V2[0] + CHUNKS_V2[1]
    bank8head = nc.dram_tensor(
        "bank8head", [P, KCH * head_cols], mybir.dt.float8e4,
        kind="ExternalInput",
    )
    vals = nc.dram_tensor("vals", [n_shard, B], mybir.dt.uint8, kind="ExternalOutput")

    DR = mybir.MatmulPerfMode.DoubleRow

    # PE warmup operands outside the Tile world: raw (uninitialized) SBUF as
    # garbage source, and a PSUM bank that is freed back to the stack before
    # the tile pools claim all 8 banks.  Safe because the in-order PE queue
    # runs every warmup MM before the first real MM, whose start=True clears
    # the bank's has_written state.
    wsrc = nc.alloc_sbuf_tensor("warm_src", [P, 2, QW], mybir.dt.float8e4)
    wsrc_ap = wsrc[:, :, :]

    # Gate-critical loads issued as raw bass BEFORE the TileContext entry
    # barrier, so their ring transfers start ~1.3 us earlier than any
    # tile-managed DMA could: featT halves on the sync ring, chunk0 on the
    # scalar ring.  Completion is signalled on manual semaphores; the waits
    # are attached to the first consuming matmuls AFTER the Tile scheduler
    # runs (it cannot model producers outside its block).
    cw0, cw1 = CHUNKS_V2[0], CHUNKS_V2[1]
    fq0_r = nc.alloc_sbuf_tensor("fq0_r", [P, KCH, QW], mybir.dt.float8e4)
    fq1_r = nc.alloc_sbuf_tensor("fq1_r", [P, KCH, QW], mybir.dt.float8e4)
    c0_r = nc.alloc_sbuf_tensor("c0_r", [P, KCH, cw0], mybir.dt.float8e4)
    s_fq0 = nc.alloc_semaphore("s_fq0")
    s_fq1 = nc.alloc_semaphore("s_fq1")
    s_c0 = nc.alloc_semaphore("s_c0")
    nc.sync.dma_start(fq0_r[:, :, :], featT8[:, 0, :, :]).then_inc(s_fq0, 16)
    nc.sync.dma_start(fq1_r[:, :, :], featT8[:, 1, :, :]).then_inc(s_fq1, 16)
    nc.scalar.dma_start(
        c0_r[:, :, :], bank8head[:, 0:KCH * cw0]
    ).then_inc(s_c0, 16)
    # Placeholder waits (sem >= 0, trivially satisfied so the Tile
    # scheduler's simulator never blocks on them); upgraded to >= 16 after
    # scheduling.  In-order engine queues make one wait per sem sufficient.
    gate_waits = []  # (BassInstruction placeholder, sem)
    defer_dmas = []  # chunk-prefetch DMAs held until the gate completes

    with tile.TileContext(nc) as tc:
        with nc.psum_tensor([P, QW], mybir.dt.float32) as wps:
            wps_ap = wps[:, :]
            for _ in range(N_WARM_MM):
                nc.tensor.matmul(
                    wps_ap, wsrc_ap[:, :, 0:P], wsrc_ap,
                    start=True, stop=True, perf_mode=DR,
                )
            gate_waits.append((nc.tensor.wait_ge(s_c0, 0), s_c0))
            gate_waits.append((nc.tensor.wait_ge(s_fq0, 0), s_fq0))
        with (
            tc.tile_pool(name="feat", bufs=1) as feat_pool,
            tc.tile_pool(name="b256", bufs=1) as b256_pool,
            tc.tile_pool(name="b512", bufs=1) as b512_pool,
            tc.tile_pool(name="b1024", bufs=1) as b1024_pool,
            tc.tile_pool(name="bankp", bufs=2) as bank_pool,
            tc.tile_pool(name="valp", bufs=6) as val_pool,
            tc.tile_pool(name="psum", bufs=4, space=bass.MemorySpace.PSUM) as psum_pool,
        ):
            # All loads go on the sync HWDGE ring in gate-priority order
            # (FIFO per ring): featT qh0, bank chunk0, featT qh1, chunks 1+.
            # Stores use the scalar ring so they never queue ahead of loads.
            # Distinct callsites: same-line pool.tile() calls share a tag and
            # would alias the same buffer (deadlock).
            small_pools = {256: b256_pool, 512: b512_pool, 1024: b1024_pool}
            bank_tiles = {}

            def new_ps():
                # Shared callsite -> one pool tag -> one rotating buffer set.
                ps = psum_pool.tile([P, B], mybir.dt.float32)
                return ps

            def new_val():
                val_t = val_pool.tile([P, B], mybir.dt.uint8)
                return val_t

            bank_tiles[0] = c0_r
            featq = [fq0_r, fq1_r]

            bank_c1 = small_pools[cw1].tile([P, KCH, cw1], mybir.dt.float8e4)
            bank_tiles[1] = bank_c1
            defer_dmas.append(nc.scalar.dma_start(
                bank_c1[:, :, :], bank8head[:, KCH * cw0:KCH * (cw0 + cw1)]
            ))

            n_subtiles = sum((cw + P - 1) // P for cw in CHUNKS_V2)
            sub_i = 0
            c0 = 0
            for ci, cw in enumerate(CHUNKS_V2):
                if ci <= 1:
                    bank_sb = bank_tiles[ci]
                else:
                    if ci < 3:
                        ca = cw
                        pool = small_pools[cw]
                    else:
                        ca = CH_ALLOC_V2
                        pool = bank_pool
                    bank_sb = pool.tile([P, KCH, ca], mybir.dt.float8e4)
                    # One DMA per chunk (plane-strided source); the HW
                    # splits it across all 16 SDMA engines.
                    dma = nc.scalar.dma_start(
                        bank_sb[:, :, :cw], bank8[:, :, c0:c0 + cw]
                    )
                    if ci == 2:
                        defer_dmas.append(dma)
                for si in range(0, cw, P):
                    sw = min(P, cw - si)
                    sub_i += 1
                    if sub_i < n_subtiles:
                        ps = new_ps()
                        val_t = new_val()
                        for qh in range(2):
                            if ci == 0 and si == 0 and qh == 1:
                                gate_waits.append(
                                    (nc.tensor.wait_ge(s_fq1, 0), s_fq1)
                                )
                            for kk in range(KK):
                                nc.tensor.matmul(
                                    ps[:sw, qh * QW:(qh + 1) * QW],
                                    bank_sb[:, 2 * kk:2 * kk + 2, si:si + sw],
                                    featq[qh][:, 2 * kk:2 * kk + 2, :],
                                    start=(kk == 0),
                                    stop=(kk == KK - 1),
                                    perf_mode=DR,
                                )
                        # (sim - 64) clamped at 0, cast to u8 — single DVE op.
                        nc.vector.tensor_scalar(
                            out=val_t[:sw, :],
                            in0=ps[:sw, :],
                            scalar1=-VAL_OFF,
                            scalar2=0.0,
                            op0=mybir.AluOpType.add,
                            op1=mybir.AluOpType.max,
                        )
                        nc.sync.dma_start(
                            vals[c0 + si:c0 + si + sw, :], val_t[:sw, :]
                        )
                    else:
                        # Final subtile: separate psum tiles per query half so
                        # the qh0 DVE+store drain while the qh1 MMs still run
                        # (a shared tile would add a whole-tile WAR edge).
                        ps_a = new_ps()
                        ps_b = new_ps()
                        val_a = new_val()
                        val_b = new_val()
                        for qh, ps_h, val_h in ((0, ps_a, val_a), (1, ps_b, val_b)):
                            for kk in range(KK):
                                nc.tensor.matmul(
                                    ps_h[:sw, 0:QW],
                                    bank_sb[:, 2 * kk:2 * kk + 2, si:si + sw],
                                    featq[qh][:, 2 * kk:2 * kk + 2, :],
                                    start=(kk == 0),
                                    stop=(kk == KK - 1),
                                    perf_mode=DR,
                                )
                            qs = slice(qh * QW, (qh + 1) * QW)
                            nc.vector.tensor_scalar(
                                out=val_h[:sw, 0:QW], in0=ps_h[:sw, 0:QW],
                                scalar1=-VAL_OFF, scalar2=0.0,
                                op0=mybir.AluOpType.add, op1=mybir.AluOpType.max,
                            )
                            eng = nc.scalar if qh == 0 else nc.sync
                            eng.dma_start(
                                vals[c0 + si:c0 + si + sw, qs], val_h[:sw, 0:QW]
                            )
                c0 += cw

    # Upgrade the placeholder waits now that the Tile scheduler has run (it
    # cannot model producers outside its block and would report a deadlock).
    for w, sem in gate_waits:
        w.wait_op(sem, 16, "sem-ge")
    # Hold the chunk1/chunk2 prefetches until the gate set has landed: the
    # two NeuronCores of an HBM stack share ~716 GB/s, so an eager prefetch
    # on one core starves its neighbour's gate transfers (seen as a +3 us
    # late start).  Later chunks queue behind chunk2 on the scalar ring.
    for dma in defer_dmas:
        dma.wait_op(s_c0, 16, "sem-ge")

    nc.compile()
    return nc


def _build_program_fp8_v0(n_shard: int = N_SHARD):
    """Previous fp8 layout (unpacked inputs, 8 DMAs per chunk). Fallback."""
    nc = bacc.Bacc("TRN2", target_bir_lowering=False, debug=False)

    featT8 = nc.dram_tensor("featT8", [D, B], mybir.dt.float8e4, kind="ExternalInput")
    bank8 = nc.dram_tensor("bank8", [D, n_shard], mybir.dt.float8e4, kind="ExternalInput")
    vals = nc.dram_tensor("vals", [n_shard, B], mybir.dt.uint8, kind="ExternalOutput")

    CH8 = 1024  # bank cols per DMA chunk (1 KiB fp8 rows)
    with tile.TileContext(nc) as tc:
        with (
            tc.tile_pool(name="feat", bufs=1) as feat_pool,
            tc.tile_pool(name="bankp", bufs=4) as bank_pool,
            tc.tile_pool(name="valp", bufs=8) as val_pool,
            tc.tile_pool(name="psum", bufs=4, space=bass.MemorySpace.PSUM) as psum_pool,
        ):
            featT_sb = feat_pool.tile([P, KK, 2, B], mybir.dt.float8e4)
            for kk in range(KK):
                for i in range(2):
                    r0 = (2 * kk + i) * P
                    nc.sync.dma_start(featT_sb[:, kk, i, :], featT8[r0:r0 + P, :])

            nch = (n_shard + CH8 - 1) // CH8
            for ci in range(nch):
                c0 = ci * CH8
                cw = min(CH8, n_shard - c0)
                bank_sb = bank_pool.tile([P, KK, 2, CH8], mybir.dt.float8e4)
                for kk in range(KK):
                    for i in range(2):
                        r0 = (2 * kk + i) * P
                        nc.sync.dma_start(
                            bank_sb[:, kk, i, :cw], bank8[r0:r0 + P, c0:c0 + cw]
                        )
                for si in range(0, cw, P):
                    sw = min(P, cw - si)
                    val_t = val_pool.tile([P, B], mybir.dt.uint8)
                    ps = psum_pool.tile([P, B], mybir.dt.float32)  # 2 PSUM banks
                    for qh in range(B // QW):
                        for kk in range(KK):
                            nc.tensor.matmul(
                                ps[:sw, qh * QW:(qh + 1) * QW],
                                bank_sb[:, kk, :, si:si + sw],
                                featT_sb[:, kk, :, qh * QW:(qh + 1) * QW],
                                start=(kk == 0),
                                stop=(kk == KK - 1),
                                perf_mode=mybir.MatmulPerfMode.DoubleRow,
                            )
                    nc.vector.tensor_scalar(
                        out=val_t[:sw, :],
                        in0=ps[:sw, :],
                        scalar1=-VAL_OFF,
                        scalar2=0.0,
                        op0=mybir.AluOpType.add,
                        op1=mybir.AluOpType.max,
                    )
                    nc.sync.dma_start(vals[c0 + si:c0 + si + sw, :], val_t[:sw, :])

    nc.compile()
    return nc


_PROGRAM_CACHE = {}


def _get_program(impl, n_shard):
    key = (impl, n_shard)
    if key not in _PROGRAM_CACHE:
        build = {
            "fp8": _build_program_fp8,
            "fp8v0": _build_program_fp8_v0,
            "bf16": _build_program_bf16,
        }[impl]
        _PROGRAM_CACHE[key] = build(n_shard)
    return _PROGRAM_CACHE[key]


def _profile_ctx():
    import contextlib

    @contextlib.contextmanager
    def _maybe_profile():
        """Optional NTFF capture via the axon NRT-profile C ABI."""
        prof_dir = os.environ.get("KNN_PROFILE_DIR")
        if not prof_dir:
            yield
            return
        import ctypes
        lib = ctypes.CDLL("/opt/axon/libaxon_pjrt.so")
        lib.axon_start_nrt_profile.argtypes = [
            ctypes.POINTER(ctypes.c_int64), ctypes.c_size_t]
        lib.axon_start_nrt_profile.restype = ctypes.c_int64
        lib.axon_stop_nrt_profile.argtypes = [ctypes.c_char_p]
        lib.axon_stop_nrt_profile.restype = ctypes.c_int64
        import jax
        jax.devices()
        rc = lib.axon_start_nrt_profile(None, 0)
        if rc != 0:
            raise RuntimeError(f"axon_start_nrt_profile rc={rc}")
        try:
            yield
        finally:
            n = lib.axon_stop_nrt_profile(str(prof_dir).encode())
            print(f"ntff profile: {n} file(s) -> {prof_dir}", flush=True)

    return _maybe_profile()


def _run_spmd(nc, in_maps):
    global LAST_EXEC_TIME_NS
    with _profile_ctx():
        res = run_bass_kernel_spmd(
            nc, in_maps, core_ids=list(range(N_CORES)), trace=False
        )
    LAST_EXEC_TIME_NS = res.exec_time_ns
    _tlog("device run done")
    return res


def _candidate_pairs_bf16(feature, bank_f32):
    """bf16+mask path: device mask -> all candidate pairs."""
    n = bank_f32.shape[1]
    n_shard = n // N_CORES
    nc = _get_program("bf16", n_shard)
    _tlog("program built")

    featT_bf = np.ascontiguousarray(feature.T).astype(ml_dtypes.bfloat16)
    bank_bf = bank_f32.astype(ml_dtypes.bfloat16)
    in_maps = [
        {
            "featT": featT_bf,
            "bank": np.ascontiguousarray(bank_bf[:, i * n_shard:(i + 1) * n_shard]),
        }
        for i in range(N_CORES)
    ]
    res = _run_spmd(nc, in_maps)
    mask = np.concatenate([res.results[i]["mask"] for i in range(N_CORES)], axis=0)

    nidx, qidx = np.nonzero(mask)  # [N, B]: sorted by bank idx
    order = np.argsort(qidx, kind="stable")  # per-query segments, nidx ascending
    qidx = qidx[order]
    nidx = nidx[order]
    counts = np.bincount(qidx, minlength=feature.shape[0])
    starts = np.zeros(feature.shape[0] + 1, dtype=np.int64)
    np.cumsum(counts, out=starts[1:])
    _tlog(f"candidates built ({len(nidx)} pairs)")
    return qidx, nidx, starts


def _candidate_pairs_fp8(feature, bank_f32, k, impl="fp8"):
    """fp8+values path: threshold, then keep only the top-k confidence window."""
    global LAST_DEV_VALS
    n = bank_f32.shape[1]
    n_shard = n // N_CORES
    nc = _get_program(impl, n_shard)
    _tlog("program built")

    f8 = feature.astype(ml_dtypes.float8_e4m3)
    bank_8 = bank_f32.astype(ml_dtypes.float8_e4m3)
    if impl == "fp8":
        # Packed layouts (see _build_program_fp8 docstring).
        featT_pack = np.ascontiguousarray(
            f8.T.reshape(KCH, P, 2, QW).transpose(1, 2, 0, 3)
        )

        def _pack_bank(shard):
            # d-row r -> partition r % 128, plane r // 128.
            return np.ascontiguousarray(
                shard.reshape(KCH, P, n_shard).transpose(1, 0, 2)
            )

        def _pack_bank_head(shard):
            # chunks 0+1 duplicated contiguous-per-partition (big DMA lines).
            cw0, cw1 = CHUNKS_V2[0], CHUNKS_V2[1]
            out = np.empty((P, KCH * (cw0 + cw1)), dtype=shard.dtype)
            out[:, :KCH * cw0] = (
                shard[:, :cw0].reshape(KCH, P, cw0)
                .transpose(1, 0, 2).reshape(P, KCH * cw0)
            )
            out[:, KCH * cw0:] = (
                shard[:, cw0:cw0 + cw1].reshape(KCH, P, cw1)
                .transpose(1, 0, 2).reshape(P, KCH * cw1)
            )
            return out

        in_maps = [
            {
                "featT8": featT_pack,
                "bank8": _pack_bank(bank_8[:, i * n_shard:(i + 1) * n_shard]),
                "bank8head": _pack_bank_head(
                    bank_8[:, i * n_shard:(i + 1) * n_shard]
                ),
            }
            for i in range(N_CORES)
        ]
    else:
        featT_8 = np.ascontiguousarray(f8.T)
        in_maps = [
            {
                "featT8": featT_8,
                "bank8": np.ascontiguousarray(bank_8[:, i * n_shard:(i + 1) * n_shard]),
            }
            for i in range(N_CORES)
        ]
    _tlog("inputs packed")
    res = _run_spmd(nc, in_maps)
    vals = np.concatenate([res.results[i]["vals"] for i in range(N_CORES)], axis=0)
    LAST_DEV_VALS = vals  # [N, B] u8: clamp(sim - VAL_OFF, 0, 255)

    m = vals >= np.uint8(T0_FP8_U8)
    nidx, qidx = np.nonzero(m)
    # The DVE f32->u8 cast rounds-to-nearest, so stored+VAL_OFF is already the
    # quantization-interval midpoint (E_FP8 covers the +-0.5 either way).
    dv = vals[nidx, qidx].astype(np.float32) + np.float32(VAL_OFF)
    order = np.argsort(qidx, kind="stable")  # per-query segments, nidx ascending
    qidx = qidx[order]
    nidx = nidx[order]
    dv = dv[order]
    b = feature.shape[0]
    counts = np.bincount(qidx, minlength=b)
    starts_all = np.zeros(b + 1, dtype=np.int64)
    np.cumsum(counts, out=starts_all[1:])
    _tlog(f"thresholded ({len(nidx)} pairs)")

    # Per query, keep only candidates that can possibly be in the true top-k:
    # dev >= dev_rank_k - 2E (see module docstring for the bound).
    keep = np.zeros(len(nidx), dtype=bool)
    for q in range(b):
        s, e = starts_all[q], starts_all[q + 1]
        c = e - s
        if c < k:
            keep[s:e] = True  # top-k loop will take the full-row fallback
            continue
        seg = dv[s:e]
        rk = np.partition(seg, c - k)[c - k]
        keep[s:e] = seg >= rk - 2.0 * E_FP8
    qidx = qidx[keep]
    nidx = nidx[keep]
    counts = np.bincount(qidx, minlength=b)
    starts = np.zeros(b + 1, dtype=np.int64)
    np.cumsum(counts, out=starts[1:])
    _tlog(f"windowed ({len(nidx)} pairs)")
    return qidx, nidx, starts


def _finish(feature, bank_f32, labels, num_classes, k, cand):
    """Exact fp32 re-rank of candidate pairs + reference post-processing.

    cand is (qidx, nidx, starts) or None (full host fallback).
    """
    b, d = feature.shape
    n = bank_f32.shape[1]

    if cand is not None:
        qidx, nidx, starts = cand
        bankT = np.ascontiguousarray(bank_f32.T)  # contiguous row gathers
        _tlog("bankT transpose done")
        vals = np.empty(len(nidx), dtype=np.float32)
        CHP = 1 << 16
        for s in range(0, len(nidx), CHP):
            e = min(s + CHP, len(nidx))
            vals[s:e] = np.einsum(
                "ij,ij->i", feature[qidx[s:e]], bankT[nidx[s:e]]
            )
        _tlog(f"exact vals done ({len(nidx)} pairs)")

    full_rows = None
    full_q0 = 0
    all_idx = np.arange(n)

    sel_q = np.empty(b * k, dtype=np.int64)
    sel_lab = np.empty(b * k, dtype=np.int64)
    sel_val = np.empty(b * k, dtype=np.float32)
    pos = 0
    ROWBLK = 64
    for q in range(b):
        if cand is not None and starts[q + 1] - starts[q] >= k:
            s, e = starts[q], starts[q + 1]
            v = vals[s:e]
            idx = nidx[s:e]
        else:
            # Exact full row (no device pre-filter, or threshold miss).
            if full_rows is None or not (full_q0 <= q < full_q0 + ROWBLK):
                full_q0 = q
                hi = min(q + ROWBLK, b)
                full_rows = feature[q:hi] @ bank_f32
            v = full_rows[q - full_q0]
            idx = all_idx
        # jax.lax.top_k semantics: descending, ties -> lower index first.
        order = np.argsort(-v, kind="stable")[:k]
        sel_q[pos:pos + k] = q
        sel_lab[pos:pos + k] = labels[idx[order]]
        sel_val[pos:pos + k] = v[order]
        pos += k
    _tlog("per-query topk done")

    with np.errstate(over="ignore"):
        w = np.exp(sel_val / np.float32(KNN_T)).astype(np.float32)
    scores = np.zeros((b, num_classes), dtype=np.float32)
    np.add.at(scores, (sel_q, sel_lab), w)
    _tlog("scatter done")
    return scores


def kernel(feature, feature_bank, feature_labels, num_classes, knn_k):
    _tlog("kernel() start")
    feature = np.asarray(feature, dtype=np.float32)
    bank_f32 = np.asarray(feature_bank, dtype=np.float32)
    labels = np.asarray(feature_labels)
    c = int(np.asarray(num_classes))
    k = int(np.asarray(knn_k))

    b, d = feature.shape
    n = bank_f32.shape[1]

    impl = os.environ.get("KNN_IMPL", "fp8")
    use_device = d == D and b == B and n % N_CORES == 0 and n // N_CORES > 0
    if use_device and impl == "fp8" and n // N_CORES != sum(CHUNKS_V2):
        impl = "fp8v0"  # packed layout is hardcoded for the spec shard size
    if use_device:
        if impl in ("fp8", "fp8v0"):
            cand = _candidate_pairs_fp8(feature, bank_f32, k, impl)
        else:
            cand = _candidate_pairs_bf16(feature, bank_f32)
    else:
        cand = None  # degenerate fallback: host does it all

    scores = _finish(feature, bank_f32, labels, c, k, cand)
    pred = np.argsort(-scores, axis=1, kind="stable").astype(np.int32)
    _tlog("final argsort done")
    return pred


# revision 36
# speedup vs baseline: 1.0258x; 1.0057x over previous
"""Distributed kNN classifier for Trainium2 (8 NeuronCores).

Strategy
--------
reference(...) computes sim = feature @ feature_bank  [B, N], takes top-k
(k=200) per query, exp(sim/0.1) weights, scatter-adds into per-class scores
and returns the descending stable argsort of those scores.

The heavy part is the [1024, 1024] @ [1024, 100000] matmul plus top-k.
feature_bank is sharded along N across the 8 cores (12500 cols each).

Device (default, fp8): each core computes its sim shard with an fp8e4m3
DoubleRow matmul (fp32 PSUM accumulation, 2 MACs/cell/cycle) and writes
uint8 `clamp(round(sim - 64), 0, 255)` — candidate mask and coarse value in
one byte.  Sims are ~N(0, 32^2); every query's true 200th-largest sim is
>= ~84, and the fp8 matmul error is bounded by E_FP8, so the candidates
with stored value >= 10 (sim >~ 74) are a guaranteed superset of the true
top-k.  The host then (a) keeps, per query, only candidates
within 2*E of the device-value 200th-largest (a confidence window that
provably contains the true top-k), (b) recomputes exact fp32 similarities
for those ~0.4% of pairs, (c) selects the exact top-k with jax.lax.top_k
tie semantics and replicates the reference's exp/scatter/argsort in numpy.
If any query yields fewer than k candidates, the host falls back to an
exact full-row recompute for it, so correctness never depends on the
threshold.

Device-kernel layout: inputs are host-packed so d-row r lives on
partition r%128, plane r//128.  featT is two [128, 8, 512] query-halves
(one DMA each on the sync HWDGE ring); the bank shard is loaded in
ramp-up chunks (256/512/1024 then 2048-wide, one DMA per chunk on the
scalar HWDGE ring) so the first matmul only gates on ~0.75 MiB.  The
first two chunks are duplicated in a contiguous-per-partition side
input (bank8head) for wide DMA lines; later chunks read the
plane-strided bank8 so their packets stay small and never starve the
featT transfers.  During the DMA prologue, N_WARM_MM dummy matmuls on
raw uninitialized SBUF keep the PE busy through the HAM activity window
(cold K=4/8 -> warm K=8/8), so the real MM stream runs at the warm
216 ns/MM roofline pace from its first instruction.  The final subtile
uses separate per-query-half PSUM/val tiles and stores on both HWDGE
rings so its qh0 postprocessing drains while the qh1 matmuls still run.
Measured: ~169.5 us of back-to-back matmuls at the fp8 DoubleRow
roofline + ~13 us DMA-gated start + ~12 us fixed framework
preamble/teardown => ~195 us/core (baseline was ~202 us).

A bf16 variant (KNN_IMPL=bf16) with a uint8 `sim > T0` mask output is kept
as a fallback, as is the unpacked fp8 layout (KNN_IMPL=fp8v0).
"""

import os
import sys
import time
import numpy as np
import ml_dtypes


def _tlog(msg, _t=[None]):
    if os.environ.get("KNN_TIMING"):
        now = time.time()
        dt = 0.0 if _t[0] is None else now - _t[0]
        _t[0] = now
        print(f"[knn +{dt:6.2f}s] {msg}", file=sys.stderr, flush=True)


import concourse.bass as bass
import concourse.bacc as bacc
import concourse.mybir as mybir
from concourse import tile
from concourse.bass_utils import run_bass_kernel_spmd

# Problem geometry (hardcoded per spec).
B = 1024          # queries
D = 1024          # feature dim
N_TOTAL = 100000  # bank size
N_CORES = 8
N_SHARD = N_TOTAL // N_CORES  # 12500

P = 128           # partitions
KCH = D // P      # 8 contraction planes (one per 128 d-rows)
KK = D // (2 * P)  # 4 DoubleRow contraction chunks (fp8)
QW = 512          # rhs free width per matmul (one PSUM bank of fp32)
CH = 512          # bank columns loaded per DMA chunk (bf16 path)

T0 = 80.0         # bf16 mask threshold (true 200th-largest sim is >= ~84.2)
VAL_OFF = 64.0    # u8 value-output offset: stored = clamp(sim - 64, 0, 255)
T0_FP8_U8 = 10    # u8 threshold (sim >~ 74; fp8 |err| <= ~6.6, margin ~10)
E_FP8 = 8.5       # fp8 matmul + u8 quantization error bound for the window

# fp8 kernel chunking: ramp-up chunks so matmuls start as soon as ~0.75 MiB
# has landed and each later chunk arrives before compute reaches it
# (sum == N_SHARD).
CHUNKS_V2 = (256, 512, 1024, 2048, 2048, 2048, 2048, 2048, 468)
CH_ALLOC_V2 = 2048  # SBUF tile width for big chunks (plane stride, 16-aligned)
N_WARM_MM = 14      # dummy MMs to warm the PE HAM gate during the DMA prologue

KNN_T = 0.1

LAST_EXEC_TIME_NS = None
LAST_DEV_VALS = None  # [N, B] device sims (fp8 path), for diagnostics


def _build_program_bf16(n_shard: int = N_SHARD):
    """bf16 matmul; uint8 mask output."""
    nc = bacc.Bacc("TRN2", target_bir_lowering=False, debug=False)

    featT = nc.dram_tensor("featT", [D, B], mybir.dt.bfloat16, kind="ExternalInput")
    bank = nc.dram_tensor("bank", [D, n_shard], mybir.dt.bfloat16, kind="ExternalInput")
    mask = nc.dram_tensor("mask", [n_shard, B], mybir.dt.uint8, kind="ExternalOutput")

    with tile.TileContext(nc) as tc:
        with (
            tc.tile_pool(name="feat", bufs=1) as feat_pool,
            tc.tile_pool(name="bankp", bufs=4) as bank_pool,
            tc.tile_pool(name="maskp", bufs=6) as mask_pool,
            tc.tile_pool(name="psum", bufs=6, space=bass.MemorySpace.PSUM) as psum_pool,
        ):
            # All of feature^T stays resident: [128, 8, 1024] bf16 (16 KiB/part)
            featT_sb = feat_pool.tile([P, KCH, B], mybir.dt.bfloat16)
            for kc in range(KCH):
                nc.sync.dma_start(featT_sb[:, kc, :], featT[kc * P:(kc + 1) * P, :])

            nch = (n_shard + CH - 1) // CH
            for ci in range(nch):
                c0 = ci * CH
                cw = min(CH, n_shard - c0)
                bank_sb = bank_pool.tile([P, KCH, CH], mybir.dt.bfloat16)
                for kc in range(KCH):
                    nc.sync.dma_start(
                        bank_sb[:, kc, :cw], bank[kc * P:(kc + 1) * P, c0:c0 + cw]
                    )
                for si in range(0, cw, P):
                    sw = min(P, cw - si)
                    mask_t = mask_pool.tile([P, B], mybir.dt.uint8)
                    for qh in range(B // QW):
                        ps = psum_pool.tile([P, QW], mybir.dt.float32)
                        for kc in range(KCH):
                            nc.tensor.matmul(
                                ps[:sw, :],
                                bank_sb[:, kc, si:si + sw],
                                featT_sb[:, kc, qh * QW:(qh + 1) * QW],
                                start=(kc == 0),
                                stop=(kc == KCH - 1),
                            )
                        nc.vector.tensor_scalar(
                            out=mask_t[:sw, qh * QW:(qh + 1) * QW],
                            in0=ps[:sw, :],
                            scalar1=T0,
                            scalar2=None,
                            op0=mybir.AluOpType.is_gt,
                        )
                    nc.sync.dma_start(mask[c0 + si:c0 + si + sw, :], mask_t[:sw, :])

    nc.compile()
    return nc


def _build_program_fp8(n_shard: int = N_SHARD):
    """fp8e4m3 DoubleRow matmul, host-packed layout (v2).

    Inputs (packed on host; d-row r -> partition r % 128, plane r // 128):
      featT8 [128, 2, 8, 512]: featT8[p, qh, j, c] = feature[512*qh + c, 128*j + p]
      bank8  [128, 8 * n_shard]: chunk-major — for chunk (c0, cw), partition p
        holds the chunk's 8 planes contiguously ([8, cw] row-major), so each
        chunk is one contiguous-per-partition DMA with >= 2 KiB lines.
    Output: vals [n_shard, B] u8 = clamp(round(sim - VAL_OFF), 0, 255).
    """
    assert n_shard == sum(CHUNKS_V2)
    nc = bacc.Bacc("TRN2", target_bir_lowering=False, debug=False)

    featT8 = nc.dram_tensor(
        "featT8", [P, 2, KCH, QW], mybir.dt.float8e4, kind="ExternalInput"
    )
    bank8 = nc.dram_tensor(
        "bank8", [P, KCH, n_shard], mybir.dt.float8e4, kind="ExternalInput"
    )
    head_cols = CHUNKS_V2[0] + CHUNKS_V2[1]
    bank8head = nc.dram_tensor(
        "bank8head", [P, KCH * head_cols], mybir.dt.float8e4,
        kind="ExternalInput",
    )
    vals = nc.dram_tensor("vals", [n_shard, B], mybir.dt.uint8, kind="ExternalOutput")

    DR = mybir.MatmulPerfMode.DoubleRow

    # PE warmup operands outside the Tile world: raw (uninitialized) SBUF as
    # garbage source, and a PSUM bank that is freed back to the stack before
    # the tile pools claim all 8 banks.  Safe because the in-order PE queue
    # runs every warmup MM before the first real MM, whose start=True clears
    # the bank's has_written state.
    wsrc = nc.alloc_sbuf_tensor("warm_src", [P, 2, QW], mybir.dt.float8e4)
    wsrc_ap = wsrc[:, :, :]

    # Gate-critical loads issued as raw bass BEFORE the TileContext entry
    # barrier, so their ring transfers start ~1.3 us earlier than any
    # tile-managed DMA could: featT halves on the sync ring, chunk0 on the
    # scalar ring.  Completion is signalled on manual semaphores; the waits
    # are attached to the first consuming matmuls AFTER the Tile scheduler
    # runs (it cannot model producers outside its block).
    cw0, cw1 = CHUNKS_V2[0], CHUNKS_V2[1]
    fq0_r = nc.alloc_sbuf_tensor("fq0_r", [P, KCH, QW], mybir.dt.float8e4)
    fq1_r = nc.alloc_sbuf_tensor("fq1_r", [P, KCH, QW], mybir.dt.float8e4)
    c0_r = nc.alloc_sbuf_tensor("c0_r", [P, KCH, cw0], mybir.dt.float8e4)
    s_fq0 = nc.alloc_semaphore("s_fq0")
    s_fq1 = nc.alloc_semaphore("s_fq1")
    s_c0 = nc.alloc_semaphore("s_c0")
    nc.sync.dma_start(fq0_r[:, :, :], featT8[:, 0, :, :]).then_inc(s_fq0, 16)
    nc.sync.dma_start(fq1_r[:, :, :], featT8[:, 1, :, :]).then_inc(s_fq1, 16)
    nc.scalar.dma_start(
        c0_r[:, :, :], bank8head[:, 0:KCH * cw0]
    ).then_inc(s_c0, 16)
    # Placeholder waits (sem >= 0, trivially satisfied so the Tile
    # scheduler's simulator never blocks on them); upgraded to >= 16 after
    # scheduling.  In-order engine queues make one wait per sem sufficient.
    gate_waits = []  # (BassInstruction placeholder, sem)
    defer_dmas = []  # chunk-prefetch DMAs held until the gate completes

    with tile.TileContext(nc) as tc:
        with nc.psum_tensor([P, QW], mybir.dt.float32) as wps:
            wps_ap = wps[:, :]
            for _ in range(N_WARM_MM):
                nc.tensor.matmul(
                    wps_ap, wsrc_ap[:, :, 0:P], wsrc_ap,
                    start=True, stop=True, perf_mode=DR,
                )
            gate_waits.append((nc.tensor.wait_ge(s_c0, 0), s_c0))
            gate_waits.append((nc.tensor.wait_ge(s_fq0, 0), s_fq0))
        with (
            tc.tile_pool(name="feat", bufs=1) as feat_pool,
            tc.tile_pool(name="b256", bufs=1) as b256_pool,
            tc.tile_pool(name="b512", bufs=1) as b512_pool,
            tc.tile_pool(name="b1024", bufs=1) as b1024_pool,
            tc.tile_pool(name="bankp", bufs=2) as bank_pool,
            tc.tile_pool(name="valp", bufs=6) as val_pool,
            tc.tile_pool(name="psum", bufs=4, space=bass.MemorySpace.PSUM) as psum_pool,
        ):
            # All loads go on the sync HWDGE ring in gate-priority order
            # (FIFO per ring): featT qh0, bank chunk0, featT qh1, chunks 1+.
            # Stores use the scalar ring so they never queue ahead of loads.
            # Distinct callsites: same-line pool.tile() calls share a tag and
            # would alias the same buffer (deadlock).
            small_pools = {256: b256_pool, 512: b512_pool, 1024: b1024_pool}
            bank_tiles = {}

            def new_ps():
                # Shared callsite -> one pool tag -> one rotating buffer set.
                ps = psum_pool.tile([P, B], mybir.dt.float32)
                return ps

            def new_val():
                val_t = val_pool.tile([P, B], mybir.dt.uint8)
                return val_t

            bank_tiles[0] = c0_r
            featq = [fq0_r, fq1_r]

            bank_c1 = small_pools[cw1].tile([P, KCH, cw1], mybir.dt.float8e4)
            bank_tiles[1] = bank_c1
            defer_dmas.append(nc.scalar.dma_start(
                bank_c1[:, :, :], bank8head[:, KCH * cw0:KCH * (cw0 + cw1)]
            ))

            n_subtiles = sum((cw + P - 1) // P for cw in CHUNKS_V2)
            sub_i = 0
            c0 = 0
            for ci, cw in enumerate(CHUNKS_V2):
                if ci <= 1:
                    bank_sb = bank_tiles[ci]
                else:
                    if ci < 3:
                        ca = cw
                        pool = small_pools[cw]
                    else:
                        ca = CH_ALLOC_V2
                        pool = bank_pool
                    bank_sb = pool.tile([P, KCH, ca], mybir.dt.float8e4)
                    # One DMA per chunk (plane-strided source); the HW
                    # splits it across all 16 SDMA engines.
                    dma = nc.scalar.dma_start(
                        bank_sb[:, :, :cw], bank8[:, :, c0:c0 + cw]
                    )
                    if ci == 2:
                        defer_dmas.append(dma)
                for si in range(0, cw, P):
                    sw = min(P, cw - si)
                    sub_i += 1
                    if sub_i < n_subtiles:
                        ps = new_ps()
                        val_t = new_val()
                        for qh in range(2):
                            if ci == 0 and si == 0 and qh == 1:
                                gate_waits.append(
                                    (nc.tensor.wait_ge(s_fq1, 0), s_fq1)
                                )
                            for kk in range(KK):
                                nc.tensor.matmul(
                                    ps[:sw, qh * QW:(qh + 1) * QW],
                                    bank_sb[:, 2 * kk:2 * kk + 2, si:si + sw],
                                    featq[qh][:, 2 * kk:2 * kk + 2, :],
                                    start=(kk == 0),
                                    stop=(kk == KK - 1),
                                    perf_mode=DR,
                                )
                        # (sim - 64) clamped at 0, cast to u8 — single DVE op.
                        nc.vector.tensor_scalar(
                            out=val_t[:sw, :],
                            in0=ps[:sw, :],
                            scalar1=-VAL_OFF,
                            scalar2=0.0,
                            op0=mybir.AluOpType.add,
                            op1=mybir.AluOpType.max,
                        )
                        nc.sync.dma_start(
                            vals[c0 + si:c0 + si + sw, :], val_t[:sw, :]
                        )
                    else:
                        # Final subtile: separate psum tiles per query half so
                        # the qh0 DVE+store drain while the qh1 MMs still run
                        # (a shared tile would add a whole-tile WAR edge).
                        ps_a = new_ps()
                        ps_b = new_ps()
                        val_a = new_val()
                        val_b = new_val()
                        for qh, ps_h, val_h in ((0, ps_a, val_a), (1, ps_b, val_b)):
                            for kk in range(KK):
                                nc.tensor.matmul(
                                    ps_h[:sw, 0:QW],
                                    bank_sb[:, 2 * kk:2 * kk + 2, si:si + sw],
                                    featq[qh][:, 2 * kk:2 * kk + 2, :],
                                    start=(kk == 0),
                                    stop=(kk == KK - 1),
                                    perf_mode=DR,
                                )
                            qs = slice(qh * QW, (qh + 1) * QW)
                            nc.vector.tensor_scalar(
                                out=val_h[:sw, 0:QW], in0=ps_h[:sw, 0:QW],
                                scalar1=-VAL_OFF, scalar2=0.0,
                                op0=mybir.AluOpType.add, op1=mybir.AluOpType.max,
                            )
                            eng = nc.scalar if qh == 0 else nc.sync
                            eng.dma_start(
                                vals[c0 + si:c0 + si + sw, qs], val_h[:sw, 0:QW]
                            )
                c0 += cw

    # Upgrade the placeholder waits now that the Tile scheduler has run (it
    # cannot model producers outside its block and would report a deadlock).
    for w, sem in gate_waits:
        w.wait_op(sem, 16, "sem-ge")
    # Hold the chunk1/chunk2 prefetches until the gate set has landed: the
    # two NeuronCores of an HBM stack share ~716 GB/s, so an eager prefetch
    # on one core starves its neighbour's gate transfers (seen as a +3 us
    # late start).  Later chunks queue behind chunk2 on the scalar ring.
    for dma in defer_dmas:
        dma.wait_op(s_c0, 16, "sem-ge")

    nc.compile()
    return nc


def _build_program_fp8_v0(n_shard: int = N_SHARD):
    """Previous fp8 layout (unpacked inputs, 8 DMAs per chunk). Fallback."""
    nc = bacc.Bacc("TRN2", target_bir_lowering=False, debug=False)

    featT8 = nc.dram_tensor("featT8", [D, B], mybir.dt.float8e4, kind="ExternalInput")
    bank8 = nc.dram_tensor("bank8", [D, n_shard], mybir.dt.float8e4, kind="ExternalInput")
    vals = nc.dram_tensor("vals", [n_shard, B], mybir.dt.uint8, kind="ExternalOutput")

    CH8 = 1024  # bank cols per DMA chunk (1 KiB fp8 rows)
    with tile.TileContext(nc) as tc:
        with (
            tc.tile_pool(name="feat", bufs=1) as feat_pool,
            tc.tile_pool(name="bankp", bufs=4) as bank_pool,
            tc.tile_pool(name="valp", bufs=8) as val_pool,
            tc.tile_pool(name="psum", bufs=4, space=bass.MemorySpace.PSUM) as psum_pool,
        ):
            featT_sb = feat_pool.tile([P, KK, 2, B], mybir.dt.float8e4)
            for kk in range(KK):
                for i in range(2):
                    r0 = (2 * kk + i) * P
                    nc.sync.dma_start(featT_sb[:, kk, i, :], featT8[r0:r0 + P, :])

            nch = (n_shard + CH8 - 1) // CH8
            for ci in range(nch):
                c0 = ci * CH8
                cw = min(CH8, n_shard - c0)
                bank_sb = bank_pool.tile([P, KK, 2, CH8], mybir.dt.float8e4)
                for kk in range(KK):
                    for i in range(2):
                        r0 = (2 * kk + i) * P
                        nc.sync.dma_start(
                            bank_sb[:, kk, i, :cw], bank8[r0:r0 + P, c0:c0 + cw]
                        )
                for si in range(0, cw, P):
                    sw = min(P, cw - si)
                    val_t = val_pool.tile([P, B], mybir.dt.uint8)
                    ps = psum_pool.tile([P, B], mybir.dt.float32)  # 2 PSUM banks
                    for qh in range(B // QW):
                        for kk in range(KK):
                            nc.tensor.matmul(
                                ps[:sw, qh * QW:(qh + 1) * QW],
                                bank_sb[:, kk, :, si:si + sw],
                                featT_sb[:, kk, :, qh * QW:(qh + 1) * QW],
                                start=(kk == 0),
                                stop=(kk == KK - 1),
                                perf_mode=mybir.MatmulPerfMode.DoubleRow,
                            )
                    nc.vector.tensor_scalar(
                        out=val_t[:sw, :],
                        in0=ps[:sw, :],
                        scalar1=-VAL_OFF,
                        scalar2=0.0,
                        op0=mybir.AluOpType.add,
                        op1=mybir.AluOpType.max,
                    )
                    nc.sync.dma_start(vals[c0 + si:c0 + si + sw, :], val_t[:sw, :])

    nc.compile()
    return nc


_PROGRAM_CACHE = {}


def _get_program(impl, n_shard):
    key = (impl, n_shard)
    if key not in _PROGRAM_CACHE:
        build = {
            "fp8": _build_program_fp8,
            "fp8v0": _build_program_fp8_v0,
            "bf16": _build_program_bf16,
        }[impl]
        _PROGRAM_CACHE[key] = build(n_shard)
    return _PROGRAM_CACHE[key]


def _profile_ctx():
    import contextlib

    @contextlib.contextmanager
    def _maybe_profile():
        """Optional NTFF capture via the axon NRT-profile C ABI."""
        prof_dir = os.environ.get("KNN_PROFILE_DIR")
        if not prof_dir:
            yield
            return
        import ctypes
        lib = ctypes.CDLL("/opt/axon/libaxon_pjrt.so")
        lib.axon_start_nrt_profile.argtypes = [
            ctypes.POINTER(ctypes.c_int64), ctypes.c_size_t]
        lib.axon_start_nrt_profile.restype = ctypes.c_int64
        lib.axon_stop_nrt_profile.argtypes = [ctypes.c_char_p]
        lib.axon_stop_nrt_profile.restype = ctypes.c_int64
        import jax
        jax.devices()
        rc = lib.axon_start_nrt_profile(None, 0)
        if rc != 0:
            raise RuntimeError(f"axon_start_nrt_profile rc={rc}")
        try:
            yield
        finally:
            n = lib.axon_stop_nrt_profile(str(prof_dir).encode())
            print(f"ntff profile: {n} file(s) -> {prof_dir}", flush=True)

    return _maybe_profile()


def _run_spmd(nc, in_maps):
    global LAST_EXEC_TIME_NS
    with _profile_ctx():
        res = run_bass_kernel_spmd(
            nc, in_maps, core_ids=list(range(N_CORES)), trace=False
        )
    LAST_EXEC_TIME_NS = res.exec_time_ns
    _tlog("device run done")
    return res


def _candidate_pairs_bf16(feature, bank_f32):
    """bf16+mask path: device mask -> all candidate pairs."""
    n = bank_f32.shape[1]
    n_shard = n // N_CORES
    nc = _get_program("bf16", n_shard)
    _tlog("program built")

    featT_bf = np.ascontiguousarray(feature.T).astype(ml_dtypes.bfloat16)
    bank_bf = bank_f32.astype(ml_dtypes.bfloat16)
    in_maps = [
        {
            "featT": featT_bf,
            "bank": np.ascontiguousarray(bank_bf[:, i * n_shard:(i + 1) * n_shard]),
        }
        for i in range(N_CORES)
    ]
    res = _run_spmd(nc, in_maps)
    mask = np.concatenate([res.results[i]["mask"] for i in range(N_CORES)], axis=0)

    nidx, qidx = np.nonzero(mask)  # [N, B]: sorted by bank idx
    order = np.argsort(qidx, kind="stable")  # per-query segments, nidx ascending
    qidx = qidx[order]
    nidx = nidx[order]
    counts = np.bincount(qidx, minlength=feature.shape[0])
    starts = np.zeros(feature.shape[0] + 1, dtype=np.int64)
    np.cumsum(counts, out=starts[1:])
    _tlog(f"candidates built ({len(nidx)} pairs)")
    return qidx, nidx, starts


def _candidate_pairs_fp8(feature, bank_f32, k, impl="fp8"):
    """fp8+values path: threshold, then keep only the top-k confidence window."""
    global LAST_DEV_VALS
    n = bank_f32.shape[1]
    n_shard = n // N_CORES
    nc = _get_program(impl, n_shard)
    _tlog("program built")

    f8 = feature.astype(ml_dtypes.float8_e4m3)
    bank_8 = bank_f32.astype(ml_dtypes.float8_e4m3)
    if impl == "fp8":
        # Packed layouts (see _build_program_fp8 docstring).
        featT_pack = np.ascontiguousarray(
            f8.T.reshape(KCH, P, 2, QW).transpose(1, 2, 0, 3)
        )

        def _pack_bank(shard):
            # d-row r -> partition r % 128, plane r // 128.
            return np.ascontiguousarray(
                shard.reshape(KCH, P, n_shard).transpose(1, 0, 2)
            )

        def _pack_bank_head(shard):
            # chunks 0+1 duplicated contiguous-per-partition (big DMA lines).
            cw0, cw1 = CHUNKS_V2[0], CHUNKS_V2[1]
            out = np.empty((P, KCH * (cw0 + cw1)), dtype=shard.dtype)
            out[:, :KCH * cw0] = (
                shard[:, :cw0].reshape(KCH, P, cw0)
                .transpose(1, 0, 2).reshape(P, KCH * cw0)
            )
            out[:, KCH * cw0:] = (
                shard[:, cw0:cw0 + cw1].reshape(KCH, P, cw1)
                .transpose(1, 0, 2).reshape(P, KCH * cw1)
            )
            return out

        in_maps = [
            {
                "featT8": featT_pack,
                "bank8": _pack_bank(bank_8[:, i * n_shard:(i + 1) * n_shard]),
                "bank8head": _pack_bank_head(
                    bank_8[:, i * n_shard:(i + 1) * n_shard]
                ),
            }
            for i in range(N_CORES)
        ]
    else:
        featT_8 = np.ascontiguousarray(f8.T)
        in_maps = [
            {
                "featT8": featT_8,
                "bank8": np.ascontiguousarray(bank_8[:, i * n_shard:(i + 1) * n_shard]),
            }
            for i in range(N_CORES)
        ]
    _tlog("inputs packed")
    res = _run_spmd(nc, in_maps)
    vals = np.concatenate([res.results[i]["vals"] for i in range(N_CORES)], axis=0)
    LAST_DEV_VALS = vals  # [N, B] u8: clamp(sim - VAL_OFF, 0, 255)

    m = vals >= np.uint8(T0_FP8_U8)
    nidx, qidx = np.nonzero(m)
    # The DVE f32->u8 cast rounds-to-nearest, so stored+VAL_OFF is already the
    # quantization-interval midpoint (E_FP8 covers the +-0.5 either way).
    dv = vals[nidx, qidx].astype(np.float32) + np.float32(VAL_OFF)
    order = np.argsort(qidx, kind="stable")  # per-query segments, nidx ascending
    qidx = qidx[order]
    nidx = nidx[order]
    dv = dv[order]
    b = feature.shape[0]
    counts = np.bincount(qidx, minlength=b)
    starts_all = np.zeros(b + 1, dtype=np.int64)
    np.cumsum(counts, out=starts_all[1:])
    _tlog(f"thresholded ({len(nidx)} pairs)")

    # Per query, keep only candidates that can possibly be in the true top-k:
    # dev >= dev_rank_k - 2E (see module docstring for the bound).
    keep = np.zeros(len(nidx), dtype=bool)
    for q in range(b):
        s, e = starts_all[q], starts_all[q + 1]
        c = e - s
        if c < k:
            keep[s:e] = True  # top-k loop will take the full-row fallback
            continue
        seg = dv[s:e]
        rk = np.partition(seg, c - k)[c - k]
        keep[s:e] = seg >= rk - 2.0 * E_FP8
    qidx = qidx[keep]
    nidx = nidx[keep]
    counts = np.bincount(qidx, minlength=b)
    starts = np.zeros(b + 1, dtype=np.int64)
    np.cumsum(counts, out=starts[1:])
    _tlog(f"windowed ({len(nidx)} pairs)")
    return qidx, nidx, starts


def _finish(feature, bank_f32, labels, num_classes, k, cand):
    """Exact fp32 re-rank of candidate pairs + reference post-processing.

    cand is (qidx, nidx, starts) or None (full host fallback).
    """
    b, d = feature.shape
    n = bank_f32.shape[1]

    if cand is not None:
        qidx, nidx, starts = cand
        bankT = np.ascontiguousarray(bank_f32.T)  # contiguous row gathers
        _tlog("bankT transpose done")
        vals = np.empty(len(nidx), dtype=np.float32)
        CHP = 1 << 16
        for s in range(0, len(nidx), CHP):
            e = min(s + CHP, len(nidx))
            vals[s:e] = np.einsum(
                "ij,ij->i", feature[qidx[s:e]], bankT[nidx[s:e]]
            )
        _tlog(f"exact vals done ({len(nidx)} pairs)")

    full_rows = None
    full_q0 = 0
    all_idx = np.arange(n)

    sel_q = np.empty(b * k, dtype=np.int64)
    sel_lab = np.empty(b * k, dtype=np.int64)
    sel_val = np.empty(b * k, dtype=np.float32)
    pos = 0
    ROWBLK = 64
    for q in range(b):
        if cand is not None and starts[q + 1] - starts[q] >= k:
            s, e = starts[q], starts[q + 1]
            v = vals[s:e]
            idx = nidx[s:e]
        else:
            # Exact full row (no device pre-filter, or threshold miss).
            if full_rows is None or not (full_q0 <= q < full_q0 + ROWBLK):
                full_q0 = q
                hi = min(q + ROWBLK, b)
                full_rows = feature[q:hi] @ bank_f32
            v = full_rows[q - full_q0]
            idx = all_idx
        # jax.lax.top_k semantics: descending, ties -> lower index first.
        order = np.argsort(-v, kind="stable")[:k]
        sel_q[pos:pos + k] = q
        sel_lab[pos:pos + k] = labels[idx[order]]
        sel_val[pos:pos + k] = v[order]
        pos += k
    _tlog("per-query topk done")

    with np.errstate(over="ignore"):
        w = np.exp(sel_val / np.float32(KNN_T)).astype(np.float32)
    scores = np.zeros((b, num_classes), dtype=np.float32)
    np.add.at(scores, (sel_q, sel_lab), w)
    _tlog("scatter done")
    return scores


def kernel(feature, feature_bank, feature_labels, num_classes, knn_k):
    _tlog("kernel() start")
    feature = np.asarray(feature, dtype=np.float32)
    bank_f32 = np.asarray(feature_bank, dtype=np.float32)
    labels = np.asarray(feature_labels)
    c = int(np.asarray(num_classes))
    k = int(np.asarray(knn_k))

    b, d = feature.shape
    n = bank_f32.shape[1]

    impl = os.environ.get("KNN_IMPL", "fp8")
    use_device = d == D and b == B and n % N_CORES == 0 and n // N_CORES > 0
    if use_device and impl == "fp8" and n // N_CORES != sum(CHUNKS_V2):
        impl = "fp8v0"  # packed layout is hardcoded for the spec shard size
    if use_device:
        if impl in ("fp8", "fp8v0"):
            cand = _candidate_pairs_fp8(feature, bank_f32, k, impl)
        else:
            cand = _candidate_pairs_bf16(feature, bank_f32)
    else:
        cand = None  # degenerate fallback: host does it all

    scores = _finish(feature, bank_f32, labels, c, k, cand)
    pred = np.argsort(-scores, axis=1, kind="stable").astype(np.int32)
    _tlog("final argsort done")
    return pred
